# revision 13
# baseline (speedup 1.0000x reference)
"""ALSH conv kernel for 8 TRN2 NeuronCores.

Strategy (data-parallel over batch, 2 images/core):
  - conv(input, kernels*mask) == conv(input, kernels) masked per output
    channel, so the heavy conv runs concurrently with the vote ->
    histogram -> AllReduce -> argmax chain; the mask lands in the epilogue.
  - Vote conv in f32 on PE (taps-in-M trick: z = A^T @ padded_img, then
    9 shifted adds), histogram via hi/lo one-hot matmuls into a 32x32
    PSUM tile, 4KB AllReduce across the 8 cores, argmax on device.
  - Main conv: per (img, och-chunk, 8-row group) PSUM tile, 9 tap
    matmuls at N=512.
"""

import sys

sys.path.insert(0, "/opt/trn_rl_repo")

import numpy as np

B, C, H, W = 16, 128, 64, 64
O = 256
KH = KW = 3
TAPS = 9
TABLE = 1024
R = 2.5
U = 0.83
M_ALSH = 9
N_CORES = 8
BPC = B // N_CORES  # images per core
HP, WP = H + 2, W + 2  # padded
PIX = HP * WP  # 4356
BIG = float(2**20)

# Main conv runs in bf16 (1 cyc/row on PE; rel-err ~4e-3, well under the
# gate). The vote path must stay f32: the argmax race is decided by 5
# votes out of 65536 and bf16 votes would flip thousands.
_CACHE = {}


def _build_consts(kernels, a, b):
    kern = np.asarray(kernels, np.float64)
    a64 = np.asarray(a, np.float64)
    b0 = float(np.asarray(b).reshape(-1)[0])

    # weights lhsT: wt[j, t, c, o'] = kernels[j*128+o', c, dy, dx]
    import ml_dtypes

    # [c, j, t, o'] so the SBUF DMA is one 4.6KB contiguous run/partition
    wt = (
        np.asarray(kernels, np.float32)
        .reshape(2, 128, C, TAPS)
        .transpose(2, 0, 3, 1)
        .astype(ml_dtypes.bfloat16)
        .copy()
    )

    arr = a64.reshape(C + 1, TAPS)
    aT = arr[:C].astype(np.float32).copy()  # [128, 9]
    a128 = arr[C]  # [9]

    # bias map: 0.5-channel contribution (with zero padding) + b, all /R
    onesp = np.zeros((HP, WP))
    onesp[1 : H + 1, 1 : W + 1] = 1.0
    bias05 = np.zeros((H, W))
    for dy in range(KH):
        for dx in range(KW):
            bias05 += 0.5 * a128[dy * 3 + dx] * onesp[dy : dy + H, dx : dx + W]
    biasR = ((bias05 + b0) / R).astype(np.float32)
    biasR2 = np.concatenate([biasR, biasR], axis=0).copy()  # [128, 64]

    iota32 = np.broadcast_to(np.arange(32, dtype=np.float32), (128, 32)).copy()
    invio = (BIG - np.arange(TABLE, dtype=np.float32)).reshape(1, TABLE).copy()

    # kernel hash buckets (f64; min boundary distance is ~4e-3, f32-safe)
    kflat = kern.reshape(O, -1)
    norms = np.linalg.norm(kflat, axis=1)
    x = kflat * (U / norms.max())
    p = (x * x).sum(1)
    powers = [p]
    for _ in range(M_ALSH - 1):
        powers.append(powers[-1] ** 2)
    P = np.concatenate([x, np.stack(powers, 1)], 1)
    h = np.floor((P @ a64 + b0) / R).astype(np.int64)
    bucket = np.abs(np.fmod(h, TABLE)).astype(np.float32)  # [256]
    bucketf = bucket.reshape(2, 128).T.copy()  # [o', j]

    return dict(
        wt=wt, aT=aT, biasR=biasR2, iota=iota32, invio=invio, bucketf=bucketf
    )


def _build_nc():
    from concourse import bacc, bass, tile
    from concourse import mybir

    F32 = mybir.dt.float32
    F32R = mybir.dt.float32r
    BF16 = mybir.dt.bfloat16
    eq = mybir.AluOpType.is_equal
    pmod = mybir.AluOpType.python_mod
    sub = mybir.AluOpType.subtract
    mul = mybir.AluOpType.mult
    add = mybir.AluOpType.add
    amax = mybir.AluOpType.abs_max
    mx = mybir.AluOpType.max

    nc = bacc.Bacc("TRN2", target_bir_lowering=False, debug=False, num_devices=N_CORES)

    t_in = nc.dram_tensor("input", [BPC, C, H, W], F32, kind="ExternalInput")
    t_inbf = nc.dram_tensor("inbf", [BPC, C, HP, WP], BF16, kind="ExternalInput")
    t_wt = nc.dram_tensor("wt", [C, 2, TAPS, 128], BF16, kind="ExternalInput")
    t_aT = nc.dram_tensor("aT", [C, TAPS], F32, kind="ExternalInput")
    t_biasR = nc.dram_tensor("biasR", [128, W], F32, kind="ExternalInput")
    t_iota = nc.dram_tensor("iota", [128, 32], F32, kind="ExternalInput")
    t_invio = nc.dram_tensor("invio", [1, TABLE], F32, kind="ExternalInput")
    t_bucketf = nc.dram_tensor("bucketf", [128, 2], F32, kind="ExternalInput")

    t_out = nc.dram_tensor("out", [BPC, O, H, W], BF16, kind="ExternalOutput")
    t_omask = nc.dram_tensor("omask", [128, 2], F32, kind="ExternalOutput")
    t_oidx = nc.dram_tensor("oindex", [1, 1], F32, kind="ExternalOutput")
    t_ocnt = nc.dram_tensor("ocount", [32, 32], F32, kind="ExternalOutput")

    with tile.TileContext(nc) as tc:
        with (
            tc.tile_pool(name="main", bufs=1) as pool,
            tc.tile_pool(name="small", bufs=1) as spool,
            tc.tile_pool(name="psum_o", bufs=4, space="PSUM") as psum_o,
            tc.tile_pool(name="psum_v", bufs=2, space="PSUM") as psum_v,
            tc.tile_pool(name="osb", bufs=4) as osb_pool,
            tc.tile_pool(name="dram", bufs=1, space="DRAM") as dram,
        ):
            # ---------------- persistent tiles / const loads ----------------
            in_raw = pool.tile([128, BPC, H, W], F32)
            wt_sb = pool.tile([128, 2, TAPS, 128], BF16)
            in_bf = pool.tile([128, BPC, HP, WP], BF16)
            aT_sb = pool.tile([C, TAPS], F32)
            biasR_sb = pool.tile([128, W], F32)
            iota_sb = pool.tile([128, 32], F32)
            invio_sb = spool.tile([1, TABLE], F32)
            bucketf_sb = pool.tile([128, 2], F32)
            z_sb = pool.tile([TAPS, BPC, HP, WP], F32)
            zs = pool.tile([128, TAPS, W], F32)  # [(img,y), tap, x]

            nc.gpsimd.dma_start(aT_sb[:], t_aT[:])
            nc.gpsimd.dma_start(biasR_sb[:], t_biasR[:])
            nc.gpsimd.dma_start(iota_sb[:], t_iota[:])
            nc.gpsimd.dma_start(invio_sb[:], t_invio[:])
            nc.gpsimd.dma_start(bucketf_sb[:], t_bucketf[:])

            # f32 vote input on the SP ring (z is the earliest consumer);
            # bf16 conv input + weights stream in parallel on the ACT ring
            for i in range(BPC):
                nc.sync.dma_start(in_raw[:, i, :, :], t_in[i, :, :, :])
            nc.scalar.dma_start(in_bf[:], t_inbf[:])
            nc.scalar.dma_start(wt_sb[:], t_wt[:])

            # ---------------- vote path (f32) ----------------
            # warm the PE clock gate with throwaway matmuls while DMAs land
            junk = pool.tile([128, 512], BF16)
            nc.vector.memset(junk[:], 0.0)
            pw = psum_v.tile([128, 512], F32, tag="pw", bufs=1)
            for w in range(30):
                nc.tensor.matmul(
                    pw[:], junk[:, :128], junk[:], start=True, stop=True
                )

            # z[t, pix] = aT[:, t] . input[:, pix]; z borders are zero
            # (pad-channel of the input is zero) so only the interior is
            # computed, from the contiguous in_raw.
            ZR = 8  # rows per chunk -> N = 512
            for i in range(BPC):
                nc.vector.memset(z_sb[:, i, 0, :], 0.0)
                nc.vector.memset(z_sb[:, i, HP - 1, :], 0.0)
                nc.vector.memset(z_sb[:, i, 1 : HP - 1, 0:1], 0.0)
                nc.vector.memset(z_sb[:, i, 1 : HP - 1, WP - 1 : WP], 0.0)
                for k in range(H // ZR):
                    pz = psum_v.tile([TAPS, ZR, W], F32, tag="pz")
                    nc.tensor.matmul(
                        pz[:],
                        aT_sb[:],
                        in_raw[:, i, k * ZR : (k + 1) * ZR, :],
                        start=True,
                        stop=True,
                    )
                    nc.any.tensor_copy(
                        z_sb[:, i, 1 + k * ZR : 1 + (k + 1) * ZR, 1 : W + 1], pz[:]
                    )

            # scatter shifted planes: zs[(i,y), t, x] = z[t, i, y+dy, x+dx]
            for i in range(BPC):
                for t in range(TAPS):
                    dy, dx = t // 3, t % 3
                    eng = nc.sync if (i * TAPS + t) % 2 == 0 else nc.scalar
                    eng.dma_start(
                        zs[i * H : (i + 1) * H, t, :],
                        z_sb[t : t + 1, i, dy : dy + H, dx : dx + W],
                    )

            # dotted = sum over taps; votes pipeline
            dotted = pool.tile([128, W], F32)
            nc.vector.tensor_reduce(
                dotted[:], zs[:].transpose([0, 2, 1]), mybir.AxisListType.X, add
            )
            t2 = pool.tile([128, W], F32)
            nc.vector.scalar_tensor_tensor(
                t2[:], dotted[:], 1.0 / R, biasR_sb[:], mul, add
            )
            # floor(x) = rne(x) - (rne(x) > x), rne via the +1.5*2^23 trick
            # (|votes| <= ~64 so the mod-1024 of the reference is a no-op)
            MAGIC = 12582912.0  # 1.5 * 2^23
            rne = pool.tile([128, W], F32)
            nc.vector.tensor_scalar(rne[:], t2[:], MAGIC, -MAGIC, add, add)
            gt = pool.tile([128, W], F32)
            nc.vector.tensor_tensor(gt[:], rne[:], t2[:], mybir.AluOpType.is_gt)
            v = pool.tile([128, W], F32)  # floor(t2) = vote
            nc.vector.tensor_tensor(v[:], rne[:], gt[:], sub)
            vb = pool.tile([128, W], F32)  # |vote| (== bucket)
            nc.scalar.activation(vb[:], v[:], mybir.ActivationFunctionType.Abs)
            s32 = pool.tile([128, W], F32)
            nc.vector.tensor_scalar_mul(s32[:], vb[:], 1.0 / 32.0)
            rne2 = pool.tile([128, W], F32)
            nc.vector.tensor_scalar(rne2[:], s32[:], MAGIC, -MAGIC, add, add)
            gt2 = pool.tile([128, W], F32)
            nc.vector.tensor_tensor(gt2[:], rne2[:], s32[:], mybir.AluOpType.is_gt)
            hi = pool.tile([128, W], F32)
            nc.vector.tensor_tensor(hi[:], rne2[:], gt2[:], sub)
            lo = pool.tile([128, W], F32)  # vb - 32*hi
            nc.vector.scalar_tensor_tensor(lo[:], hi[:], -32.0, vb[:], mul, add)

            # one-hot indicators (bf16 0/1 exact) and 64 accumulating matmuls
            u_full = pool.tile([128, W, 32], BF16)
            v_full = pool.tile([128, W, 32], BF16)
            nc.vector.tensor_tensor(
                u_full[:],
                hi[:].unsqueeze(2).broadcast_to([128, W, 32]),
                iota_sb[:].unsqueeze(1).broadcast_to([128, W, 32]),
                eq,
            )
            nc.vector.tensor_tensor(
                v_full[:],
                lo[:].unsqueeze(2).broadcast_to([128, W, 32]),
                iota_sb[:].unsqueeze(1).broadcast_to([128, W, 32]),
                eq,
            )
            pcnt = psum_v.tile([32, 32], F32, tag="pcnt", bufs=1)
            for cix in range(W):
                nc.tensor.matmul(
                    pcnt[:],
                    u_full[:, cix, :],
                    v_full[:, cix, :],
                    start=(cix == 0),
                    stop=(cix == W - 1),
                )

            # ---------------- main conv (stage A only) ----------------
            G = 8  # rows per psum tile -> N = 512
            osbs = {}
            for i in range(BPC):
                for j in range(2):
                    osb = osb_pool.tile([128, H, W], BF16, tag="osb", name=f"osb{i}{j}")
                    osbs[(i, j)] = osb
                    for g in range(H // G):
                        po = psum_o.tile([128, G, W], F32, tag="po")
                        for t in range(TAPS):
                            dy, dx = t // 3, t % 3
                            lhsT = wt_sb[:, j, t, :]
                            rhs = in_bf[:, i, g * G + dy : g * G + dy + G, dx : dx + W]
                            nc.tensor.matmul(
                                po[:], lhsT, rhs, start=(t == 0), stop=(t == TAPS - 1)
                            )
                        nc.scalar.copy(osb[:, g * G : (g + 1) * G, :], po[:])

            # ---------------- AllReduce + argmax + mask (DVE only) ----------------
            cnt_sb = spool.tile([32, 32], F32)
            nc.vector.tensor_copy(cnt_sb[:], pcnt[:])
            cc_in = dram.tile([32, 32], F32)
            cc_out = dram.tile([32, 32], F32)
            nc.gpsimd.dma_start(cc_in[:], cnt_sb[:])
            nc.gpsimd.collective_compute(
                "AllReduce",
                add,
                replica_groups=[list(range(N_CORES))],
                ins=[cc_in[:].opt()],
                outs=[cc_out[:].opt()],
            )
            gcnt1 = spool.tile([1, TABLE], F32)
            nc.sync.dma_start(gcnt1[:], cc_out[:].flatten().unsqueeze(0))
            nc.sync.dma_start(t_ocnt[:], cc_out[:])

            m1 = spool.tile([1, 1], F32)
            nc.vector.tensor_reduce(m1[:], gcnt1[:], mybir.AxisListType.X, mx)
            eqm1 = spool.tile([1, TABLE], F32)
            nc.vector.tensor_tensor(
                eqm1[:], gcnt1[:], m1[:].broadcast_to([1, TABLE]), eq
            )
            cand1 = spool.tile([1, TABLE], F32)
            nc.vector.tensor_tensor(cand1[:], eqm1[:], invio_sb[:], mul)
            cmax = spool.tile([1, 1], F32)
            nc.vector.tensor_reduce(cmax[:], cand1[:], mybir.AxisListType.X, mx)
            negidx = spool.tile([1, 1], F32)
            nc.vector.tensor_scalar(negidx[:], cmax[:], BIG, None, sub)
            idxf = spool.tile([1, 1], F32)  # argmax bucket id
            nc.vector.tensor_scalar(idxf[:], negidx[:], -1.0, None, mul)

            idx128 = pool.tile([128, 1], F32)
            nc.gpsimd.partition_broadcast(idx128[:], idxf[:])
            maskf = pool.tile([128, 2], F32)
            nc.vector.tensor_tensor(
                maskf[:], bucketf_sb[:], idx128[:].broadcast_to([128, 2]), eq
            )
            nc.sync.dma_start(t_omask[:], maskf[:])
            nc.sync.dma_start(t_oidx[:], idxf[:])

            # ---------------- stage B: mask + store ----------------
            for i in range(BPC):
                for j in range(2):
                    osb = osbs[(i, j)]
                    nc.vector.tensor_scalar(
                        osb[:], osb[:], maskf[:, j : j + 1], None, mul
                    )
                    nc.sync.dma_start(t_out[i, j * 128 : (j + 1) * 128, :, :], osb[:])
    nc.compile()
    return nc


def kernel(input, kernels, a, b):
    import ml_dtypes
    from concourse import bass_utils

    inp = np.ascontiguousarray(np.asarray(input, np.float32))
    inbf = np.zeros((B, C, HP, WP), ml_dtypes.bfloat16)
    inbf[:, :, 1 : H + 1, 1 : W + 1] = inp
    consts = _build_consts(kernels, a, b)

    if "nc" not in _CACHE:
        _CACHE["nc"] = _build_nc()
    nc = _CACHE["nc"]

    in_maps = []
    for core in range(N_CORES):
        m = {
            "input": inp[core * BPC : (core + 1) * BPC],
            "inbf": inbf[core * BPC : (core + 1) * BPC],
        }
        m.update(consts)
        in_maps.append(m)

    res = bass_utils.run_bass_kernel_spmd(
        nc, in_maps, core_ids=list(range(N_CORES))
    )
    outs = res.results

    out_full = np.concatenate(
        [outs[c]["out"].astype(np.float32) for c in range(N_CORES)], axis=0
    )
    idx = np.int32(round(float(outs[0]["oindex"][0, 0])))
    maskf = outs[0]["omask"]  # [o', j]
    mask = (maskf.T.reshape(O) > 0.5)
    return out_full, np.array(idx, np.int32), mask


# revision 14
# speedup vs baseline: 1.1523x; 1.1523x over previous
"""ALSH conv kernel for 8 TRN2 NeuronCores.

Strategy (data-parallel over batch, 2 images/core):
  - conv(input, kernels*mask) == conv(input, kernels) masked per output
    channel, so the heavy conv runs concurrently with the vote ->
    histogram -> AllReduce -> argmax chain; the mask lands in the epilogue.
  - Vote conv in f32 on PE (taps-in-M trick: z = A^T @ padded_img, then
    9 shifted adds), histogram via hi/lo one-hot matmuls into a 32x32
    PSUM tile, 4KB AllReduce across the 8 cores, argmax on device.
  - Main conv: per (img, och-chunk, 8-row group) PSUM tile, 9 tap
    matmuls at N=512.
"""

import sys

sys.path.insert(0, "/opt/trn_rl_repo")

import numpy as np

B, C, H, W = 16, 128, 64, 64
O = 256
KH = KW = 3
TAPS = 9
TABLE = 1024
R = 2.5
U = 0.83
M_ALSH = 9
N_CORES = 8
BPC = B // N_CORES  # images per core
HP, WP = H + 2, W + 2  # padded
PIX = HP * WP  # 4356
BIG = float(2**20)

# Main conv runs in bf16 (1 cyc/row on PE; rel-err ~4e-3, well under the
# gate). The vote path must stay f32: the argmax race is decided by 5
# votes out of 65536 and bf16 votes would flip thousands.
_CACHE = {}


def _build_consts(kernels, a, b):
    kern = np.asarray(kernels, np.float64)
    a64 = np.asarray(a, np.float64)
    b0 = float(np.asarray(b).reshape(-1)[0])

    # weights lhsT: wt[j, t, c, o'] = kernels[j*128+o', c, dy, dx]
    import ml_dtypes

    # [c, j, t, o'] so the SBUF DMA is one 4.6KB contiguous run/partition
    wt = (
        np.asarray(kernels, np.float32)
        .reshape(2, 128, C, TAPS)
        .transpose(2, 0, 3, 1)
        .astype(ml_dtypes.bfloat16)
        .copy()
    )

    arr = a64.reshape(C + 1, TAPS)
    aT = arr[:C].astype(np.float32).copy()  # [128, 9]
    a128 = arr[C]  # [9]

    # bias map: 0.5-channel contribution (with zero padding) + b, all /R
    onesp = np.zeros((HP, WP))
    onesp[1 : H + 1, 1 : W + 1] = 1.0
    bias05 = np.zeros((H, W))
    for dy in range(KH):
        for dx in range(KW):
            bias05 += 0.5 * a128[dy * 3 + dx] * onesp[dy : dy + H, dx : dx + W]
    biasR = ((bias05 + b0) / R).astype(np.float32)
    biasR2 = np.concatenate([biasR, biasR], axis=0).copy()  # [128, 64]

    iota32 = np.broadcast_to(np.arange(32, dtype=np.float32), (128, 32)).copy()
    invio = (BIG - np.arange(TABLE, dtype=np.float32)).reshape(1, TABLE).copy()

    # kernel hash buckets (f64; min boundary distance is ~4e-3, f32-safe)
    kflat = kern.reshape(O, -1)
    norms = np.linalg.norm(kflat, axis=1)
    x = kflat * (U / norms.max())
    p = (x * x).sum(1)
    powers = [p]
    for _ in range(M_ALSH - 1):
        powers.append(powers[-1] ** 2)
    P = np.concatenate([x, np.stack(powers, 1)], 1)
    h = np.floor((P @ a64 + b0) / R).astype(np.int64)
    bucket = np.abs(np.fmod(h, TABLE)).astype(np.float32)  # [256]
    bucketf = bucket.reshape(2, 128).T.copy()  # [o', j]

    return dict(
        wt=wt, aT=aT, biasR=biasR2, iota=iota32, invio=invio, bucketf=bucketf
    )


def _build_nc():
    from concourse import bacc, bass, tile
    from concourse import mybir

    F32 = mybir.dt.float32
    F32R = mybir.dt.float32r
    BF16 = mybir.dt.bfloat16
    eq = mybir.AluOpType.is_equal
    pmod = mybir.AluOpType.python_mod
    sub = mybir.AluOpType.subtract
    mul = mybir.AluOpType.mult
    add = mybir.AluOpType.add
    amax = mybir.AluOpType.abs_max
    mx = mybir.AluOpType.max

    nc = bacc.Bacc("TRN2", target_bir_lowering=False, debug=False, num_devices=N_CORES)

    t_in = nc.dram_tensor("input", [BPC, C, H, W], F32, kind="ExternalInput")
    t_inbf = nc.dram_tensor("inbf", [BPC, C, HP, WP], BF16, kind="ExternalInput")
    t_wt = nc.dram_tensor("wt", [C, 2, TAPS, 128], BF16, kind="ExternalInput")
    t_aT = nc.dram_tensor("aT", [C, TAPS], F32, kind="ExternalInput")
    t_biasR = nc.dram_tensor("biasR", [128, W], F32, kind="ExternalInput")
    t_iota = nc.dram_tensor("iota", [128, 32], F32, kind="ExternalInput")
    t_invio = nc.dram_tensor("invio", [1, TABLE], F32, kind="ExternalInput")
    t_bucketf = nc.dram_tensor("bucketf", [128, 2], F32, kind="ExternalInput")

    t_out = nc.dram_tensor("out", [BPC, O, H, W], BF16, kind="ExternalOutput")
    t_omask = nc.dram_tensor("omask", [128, 2], F32, kind="ExternalOutput")
    t_oidx = nc.dram_tensor("oindex", [1, 1], F32, kind="ExternalOutput")
    t_ocnt = nc.dram_tensor("ocount", [32, 32], F32, kind="ExternalOutput")

    with tile.TileContext(nc) as tc:
        with (
            tc.tile_pool(name="main", bufs=1) as pool,
            tc.tile_pool(name="small", bufs=1) as spool,
            tc.tile_pool(name="psum_o", bufs=4, space="PSUM") as psum_o,
            tc.tile_pool(name="psum_v", bufs=2, space="PSUM") as psum_v,
            tc.tile_pool(name="osb", bufs=4) as osb_pool,
            tc.tile_pool(name="dram", bufs=1, space="DRAM") as dram,
        ):
            # ---------------- persistent tiles / const loads ----------------
            in_raw = pool.tile([128, BPC, H, W], F32)
            wt_sb = pool.tile([128, 2, TAPS, 128], BF16)
            in_bf = pool.tile([128, BPC, HP, WP], BF16)
            aT_sb = pool.tile([C, TAPS], F32)
            biasR_sb = pool.tile([128, W], F32)
            iota_sb = pool.tile([128, 32], F32)
            invio_sb = spool.tile([1, TABLE], F32)
            bucketf_sb = pool.tile([128, 2], F32)
            z_sb = pool.tile([TAPS, BPC, HP, WP], F32)
            zs = pool.tile([128, TAPS, W], F32)  # [(img,y), tap, x]

            nc.gpsimd.dma_start(aT_sb[:], t_aT[:])
            nc.gpsimd.dma_start(biasR_sb[:], t_biasR[:])
            nc.gpsimd.dma_start(iota_sb[:], t_iota[:])
            nc.gpsimd.dma_start(invio_sb[:], t_invio[:])
            nc.gpsimd.dma_start(bucketf_sb[:], t_bucketf[:])

            # vote input first (z is the earliest consumer and gates the
            # histogram -> AllReduce chain); conv inputs follow on same ring
            for i in range(BPC):
                nc.sync.dma_start(in_raw[:, i, :, :], t_in[i, :, :, :])
            nc.sync.dma_start(in_bf[:], t_inbf[:])
            nc.sync.dma_start(wt_sb[:], t_wt[:])

            # ---------------- vote path (f32) ----------------
            # warm the PE clock gate with throwaway matmuls while DMAs land
            junk = pool.tile([128, 512], BF16)
            nc.vector.memset(junk[:], 0.0)
            pw = psum_v.tile([128, 512], F32, tag="pw", bufs=1)
            for w in range(16):
                nc.tensor.matmul(
                    pw[:], junk[:, :128], junk[:], start=True, stop=True
                )

            # z[t, pix] = aT[:, t] . input[:, pix]; z borders are zero
            # (pad-channel of the input is zero) so only the interior is
            # computed, from the contiguous in_raw.
            ZR = 8  # rows per chunk -> N = 512
            for i in range(BPC):
                nc.vector.memset(z_sb[:, i, 0, :], 0.0)
                nc.vector.memset(z_sb[:, i, HP - 1, :], 0.0)
                nc.vector.memset(z_sb[:, i, 1 : HP - 1, 0:1], 0.0)
                nc.vector.memset(z_sb[:, i, 1 : HP - 1, WP - 1 : WP], 0.0)
                for k in range(H // ZR):
                    pz = psum_v.tile([TAPS, ZR, W], F32, tag="pz")
                    nc.tensor.matmul(
                        pz[:],
                        aT_sb[:],
                        in_raw[:, i, k * ZR : (k + 1) * ZR, :],
                        start=True,
                        stop=True,
                    )
                    nc.any.tensor_copy(
                        z_sb[:, i, 1 + k * ZR : 1 + (k + 1) * ZR, 1 : W + 1], pz[:]
                    )

            # scatter shifted planes: zs[(i,y), t, x] = z[t, i, y+dy, x+dx]
            for i in range(BPC):
                for t in range(TAPS):
                    dy, dx = t // 3, t % 3
                    eng = nc.sync if (i * TAPS + t) % 2 == 0 else nc.scalar
                    eng.dma_start(
                        zs[i * H : (i + 1) * H, t, :],
                        z_sb[t : t + 1, i, dy : dy + H, dx : dx + W],
                    )

            # dotted = sum over taps; votes pipeline
            dotted = pool.tile([128, W], F32)
            nc.vector.tensor_reduce(
                dotted[:], zs[:].transpose([0, 2, 1]), mybir.AxisListType.X, add
            )
            t2 = pool.tile([128, W], F32)
            nc.vector.scalar_tensor_tensor(
                t2[:], dotted[:], 1.0 / R, biasR_sb[:], mul, add
            )
            # floor(x) = rne(x) - (rne(x) > x), rne via the +1.5*2^23 trick
            # (|votes| <= ~64 so the mod-1024 of the reference is a no-op)
            MAGIC = 12582912.0  # 1.5 * 2^23
            rne = pool.tile([128, W], F32)
            nc.vector.tensor_scalar(rne[:], t2[:], MAGIC, -MAGIC, add, add)
            gt = pool.tile([128, W], F32)
            nc.vector.tensor_tensor(gt[:], rne[:], t2[:], mybir.AluOpType.is_gt)
            v = pool.tile([128, W], F32)  # floor(t2) = vote
            nc.vector.tensor_tensor(v[:], rne[:], gt[:], sub)
            vb = pool.tile([128, W], F32)  # |vote| (== bucket)
            nc.scalar.activation(vb[:], v[:], mybir.ActivationFunctionType.Abs)
            s32 = pool.tile([128, W], F32)
            nc.vector.tensor_scalar_mul(s32[:], vb[:], 1.0 / 32.0)
            rne2 = pool.tile([128, W], F32)
            nc.vector.tensor_scalar(rne2[:], s32[:], MAGIC, -MAGIC, add, add)
            gt2 = pool.tile([128, W], F32)
            nc.vector.tensor_tensor(gt2[:], rne2[:], s32[:], mybir.AluOpType.is_gt)
            hi = pool.tile([128, W], F32)
            nc.vector.tensor_tensor(hi[:], rne2[:], gt2[:], sub)
            lo = pool.tile([128, W], F32)  # vb - 32*hi
            nc.vector.scalar_tensor_tensor(lo[:], hi[:], -32.0, vb[:], mul, add)

            # one-hot indicators (bf16 0/1 exact) and 64 accumulating matmuls
            u_full = pool.tile([128, W, 32], BF16)
            v_full = pool.tile([128, W, 32], BF16)
            nc.vector.tensor_tensor(
                u_full[:],
                hi[:].unsqueeze(2).broadcast_to([128, W, 32]),
                iota_sb[:].unsqueeze(1).broadcast_to([128, W, 32]),
                eq,
            )
            nc.vector.tensor_tensor(
                v_full[:],
                lo[:].unsqueeze(2).broadcast_to([128, W, 32]),
                iota_sb[:].unsqueeze(1).broadcast_to([128, W, 32]),
                eq,
            )
            pcnt = psum_v.tile([32, 32], F32, tag="pcnt", bufs=1)
            for cix in range(W):
                nc.tensor.matmul(
                    pcnt[:],
                    u_full[:, cix, :],
                    v_full[:, cix, :],
                    start=(cix == 0),
                    stop=(cix == W - 1),
                )

            # ---------------- main conv (stage A only) ----------------
            G = 8  # rows per psum tile -> N = 512
            osbs = {}
            for i in range(BPC):
                for j in range(2):
                    osb = osb_pool.tile([128, H, W], BF16, tag="osb", name=f"osb{i}{j}")
                    osbs[(i, j)] = osb
                    for g in range(H // G):
                        po = psum_o.tile([128, G, W], F32, tag="po")
                        for t in range(TAPS):
                            dy, dx = t // 3, t % 3
                            lhsT = wt_sb[:, j, t, :]
                            rhs = in_bf[:, i, g * G + dy : g * G + dy + G, dx : dx + W]
                            nc.tensor.matmul(
                                po[:], lhsT, rhs, start=(t == 0), stop=(t == TAPS - 1)
                            )
                        nc.scalar.copy(osb[:, g * G : (g + 1) * G, :], po[:])

            # ---------------- AllReduce + argmax + mask (DVE only) ----------------
            cnt_sb = spool.tile([32, 32], F32)
            nc.vector.tensor_copy(cnt_sb[:], pcnt[:])
            cc_in = dram.tile([32, 32], F32)
            cc_out = dram.tile([32, 32], F32)
            nc.gpsimd.dma_start(cc_in[:], cnt_sb[:])
            nc.gpsimd.collective_compute(
                "AllReduce",
                add,
                replica_groups=[list(range(N_CORES))],
                ins=[cc_in[:].opt()],
                outs=[cc_out[:].opt()],
            )
            gcnt1 = spool.tile([1, TABLE], F32)
            nc.sync.dma_start(gcnt1[:], cc_out[:].flatten().unsqueeze(0))
            nc.sync.dma_start(t_ocnt[:], cc_out[:])

            m1 = spool.tile([1, 1], F32)
            nc.vector.tensor_reduce(m1[:], gcnt1[:], mybir.AxisListType.X, mx)
            eqm1 = spool.tile([1, TABLE], F32)
            nc.vector.tensor_tensor(
                eqm1[:], gcnt1[:], m1[:].broadcast_to([1, TABLE]), eq
            )
            cand1 = spool.tile([1, TABLE], F32)
            nc.vector.tensor_tensor(cand1[:], eqm1[:], invio_sb[:], mul)
            cmax = spool.tile([1, 1], F32)
            nc.vector.tensor_reduce(cmax[:], cand1[:], mybir.AxisListType.X, mx)
            negidx = spool.tile([1, 1], F32)
            nc.vector.tensor_scalar(negidx[:], cmax[:], BIG, None, sub)
            idxf = spool.tile([1, 1], F32)  # argmax bucket id
            nc.vector.tensor_scalar(idxf[:], negidx[:], -1.0, None, mul)

            idx128 = pool.tile([128, 1], F32)
            nc.gpsimd.partition_broadcast(idx128[:], idxf[:])
            maskf = pool.tile([128, 2], F32)
            nc.vector.tensor_tensor(
                maskf[:], bucketf_sb[:], idx128[:].broadcast_to([128, 2]), eq
            )
            nc.sync.dma_start(t_omask[:], maskf[:])
            nc.sync.dma_start(t_oidx[:], idxf[:])

            # ---------------- stage B: mask + store ----------------
            for i in range(BPC):
                for j in range(2):
                    osb = osbs[(i, j)]
                    nc.vector.tensor_scalar(
                        osb[:], osb[:], maskf[:, j : j + 1], None, mul
                    )
                    nc.sync.dma_start(t_out[i, j * 128 : (j + 1) * 128, :, :], osb[:])
    nc.compile()
    return nc


def kernel(input, kernels, a, b):
    import ml_dtypes
    from concourse import bass_utils

    inp = np.ascontiguousarray(np.asarray(input, np.float32))
    inbf = np.zeros((B, C, HP, WP), ml_dtypes.bfloat16)
    inbf[:, :, 1 : H + 1, 1 : W + 1] = inp
    consts = _build_consts(kernels, a, b)

    if "nc" not in _CACHE:
        _CACHE["nc"] = _build_nc()
    nc = _CACHE["nc"]

    in_maps = []
    for core in range(N_CORES):
        m = {
            "input": inp[core * BPC : (core + 1) * BPC],
            "inbf": inbf[core * BPC : (core + 1) * BPC],
        }
        m.update(consts)
        in_maps.append(m)

    res = bass_utils.run_bass_kernel_spmd(
        nc, in_maps, core_ids=list(range(N_CORES))
    )
    outs = res.results

    out_full = np.concatenate(
        [outs[c]["out"].astype(np.float32) for c in range(N_CORES)], axis=0
    )
    idx = np.int32(round(float(outs[0]["oindex"][0, 0])))
    maskf = outs[0]["omask"]  # [o', j]
    mask = (maskf.T.reshape(O) > 0.5)
    return out_full, np.array(idx, np.int32), mask
